# revision 23
# baseline (speedup 1.0000x reference)
"""Trainium2 Bass kernel for nn_AttentionSE3 (graph attention message passing).

v2 strategy (feature-on-partition transposed ELL layout, TensorE reductions):
- Host sorts nodes by in-degree into 128-node blocks, deals blocks round-robin
  to 8 cores, pads each block's edge lists to the block-group capacity D.
  Per group g the slots form a [D, 128] grid (d-major): col = d*128 + node_row.
- Device tiles are TRANSPOSED: kT [128 feats, D*128], vT [104, D*128] where
  rows 0..95 are value channels (h-major: c = h*12 + cx) and rows 96..103 are
  constant 1.0 (so the weighted-value product's rows 96..103 carry the raw
  exp-weights, giving the softmax denominator for free in the same matmul).
- Per group: DVE computes w = kT * q (q broadcast over d: stride-0 middle dim,
  contiguous 128-wide inner runs -> full 2x bf16 throughput).  TensorE reduces
  over the 16 k-features of each head AND replicates each head's logit to its
  13 output rows in one matmul with a fixed block-ones lhsT [128, 104].
  ScalarE applies exp (scale folded) PSUM->SBUF.  DVE multiplies by vT.
  TensorE then segment-sums over d via D accumulating identity matmuls into a
  [104, 128] PSUM tile (rows 0..95 weighted values, 96..103 denominators).
- Softmax max-subtraction is dropped (logits ~ N(0,1): exp never overflows);
  padded slots contribute exp(0)=1 to the denominator and are corrected by a
  host-computed pad count.  Normalization happens once at the end: denominator
  rows - pad counts, reciprocal, TensorE 8->96 replication, one multiply.
"""

import numpy as np

import concourse.bacc as bacc
import concourse.mybir as mybir
from concourse import tile
from concourse.bass_utils import run_bass_kernel_spmd

try:
    import ml_dtypes
    BF16_NP = np.dtype(ml_dtypes.bfloat16)
except ImportError:  # pragma: no cover
    BF16_NP = None

N_NODES = 50000
H = 8
P = 128  # nodes per block
N_CORES = 8
ROWS = 104  # 96 value channels + 8 ones-rows (denominator trick)
SCALE = float(1.0 / np.sqrt(128.0))
F32 = mybir.dt.float32
BF16 = mybir.dt.bfloat16

# Fraction of the two big elementwise multiplies routed to GPSIMD.
GP_FRAC_K = 0.0
GP_FRAC_V = 0.0
GP_FRAC_T = 0.0  # share of the d-halving tree pass on GPSIMD
# ScalarE exp chunk width (PSUM tile cols; matmuls within are <=512)
EXP_CHUNK = 1024


# ---------------------------------------------------------------- host prep

def prepare(value, key, query0, query1, edge_index, n_nodes=N_NODES, n_cores=N_CORES):
    value = np.asarray(value, dtype=np.float32)
    key = np.asarray(key, dtype=np.float32)
    query0 = np.asarray(query0, dtype=np.float32)
    query1 = np.asarray(query1, dtype=np.float32)
    n_edges = key.shape[0]

    dst = np.asarray(edge_index[1], dtype=np.int64)
    deg = np.bincount(dst, minlength=n_nodes).astype(np.int64)
    n_pad = -(-n_nodes // (P * n_cores)) * (P * n_cores)
    deg_pad = np.concatenate([deg, np.zeros(n_pad - n_nodes, dtype=np.int64)])
    nb = n_pad // P
    ng = nb // n_cores

    order = np.argsort(deg_pad, kind="stable")  # node ids, degree-ascending
    degs_o = deg_pad[order]

    blk_max = degs_o.reshape(nb, P).max(axis=1)
    D_eff = np.maximum(blk_max.reshape(ng, n_cores).max(axis=1), 1).astype(np.int64)
    D_eff = (D_eff + 1) // 2 * 2  # even, for the d-halving tree pass
    off = np.concatenate([[0], np.cumsum(P * D_eff)]).astype(np.int64)
    S = int(off[-1])  # cols per core

    pos = np.arange(n_pad)
    block = pos // P
    g_of = block // n_cores
    core_of = block % n_cores
    row = pos % P

    edge_order = np.argsort(dst, kind="stable")
    starts = np.concatenate([[0], np.cumsum(deg)])

    pp = np.repeat(pos, degs_o)           # padded-node position per real edge
    cum0 = np.concatenate([[0], np.cumsum(degs_o)])[:-1]
    d_idx = np.arange(n_edges) - np.repeat(cum0, degs_o)
    node_of_pp = order[pp]
    edge_ids = edge_order[starts[node_of_pp] + d_idx]
    # d-major slot layout: col = off[g] + d*128 + row
    col_global = core_of[pp] * S + off[g_of[pp]] + d_idx * P + row[pp]

    dt = BF16_NP
    kp_flat = np.zeros((n_cores * S, 128), dtype=dt)
    kp_flat[col_global] = key[edge_ids]
    vp_flat = np.zeros((n_cores * S, 96), dtype=dt)
    vp_flat[col_global] = value.reshape(n_edges, 96)[edge_ids]

    qfull = np.concatenate([query0, query1], axis=-1).reshape(n_nodes, 128)
    q_pad = np.zeros((n_pad, 128), dtype=np.float32)
    q_pad[:n_nodes] = qfull

    pc = (D_eff[g_of] - degs_o[pos]).astype(np.float32)  # pad count per padded node
    zero_deg = degs_o[pos] == 0
    pc[zero_deg] = (D_eff[g_of[zero_deg]] - 1).astype(np.float32)

    ids_blocks = order.reshape(nb, P)

    in_maps = []
    for c in range(n_cores):
        kT = np.ascontiguousarray(kp_flat[c * S:(c + 1) * S].T)  # [128, S]
        vT = np.empty((ROWS, S), dtype=dt)
        vT[:96] = vp_flat[c * S:(c + 1) * S].T
        vT[96:] = np.ones((8, S), dtype=dt)
        ids_c = ids_blocks[c::n_cores]                           # [ng, 128]
        qT = np.ascontiguousarray(
            q_pad[ids_c].transpose(2, 0, 1).reshape(128, ng * P)).astype(dt)
        # pad counts packed [8, ng*P] -> [128, ng*P//16] (partition kp = k*8+p
        # holds wide[p, k*W + j]); rows p identical so only k*W+j matters.
        pc_row = pc.reshape(nb, P)[c::n_cores].reshape(ng * P)
        W = ng * P // 16
        pc_c = np.ascontiguousarray(
            np.broadcast_to(pc_row.reshape(16, 1, W), (16, 8, W))
            .reshape(128, W)).astype(np.float32)
        in_maps.append({"kt": kT, "vt": vT, "qt": qT, "pc": pc_c,
                        "cst": _make_consts()})

    meta = dict(D_eff=D_eff, off=off, S=S, NG=ng, NB=nb, order=order,
                n_nodes=n_nodes, n_pad=n_pad)
    return in_maps, meta


def _make_consts():
    """lhsT constants [128, 352] bf16: block-ones [128,128] | I128 | rep8->96.
    ones128 cols 104..127 are zero so the padded output rows are exact 0."""
    cst = np.zeros((128, 352), dtype=BF16_NP)
    pidx = np.arange(128)
    hp = pidx // 16
    for c in range(104):
        hc = c // 12 if c < 96 else c - 96
        cst[hp == hc, c] = 1.0
    cst[:, 128:256] = np.eye(128, dtype=np.float32)
    for c in range(96):
        cst[c // 12, 256 + c] = 1.0
    return cst


def unshard_output(out_cores, meta):
    """out_cores: list of [96, NG*128] f32 -> [n_nodes, 32, 3]."""
    ng, nb = meta["NG"], meta["NB"]
    n_cores = len(out_cores)
    order, n_nodes, n_pad = meta["order"], meta["n_nodes"], meta["n_pad"]
    out_sorted = np.zeros((nb, P, 96), dtype=np.float32)
    for c in range(n_cores):
        out_sorted[c::n_cores] = (
            out_cores[c].reshape(96, ng, P).transpose(1, 2, 0))
    out_sorted = out_sorted.reshape(n_pad, 96)
    out_full = np.zeros((n_nodes, 96), dtype=np.float32)
    mask = order < n_nodes
    out_full[order[mask]] = out_sorted[mask]
    return out_full.reshape(n_nodes, 32, 3)


# ---------------------------------------------------------------- bass kernel

def build(D_eff, S, NG, n_cores=N_CORES):
    D_eff = [int(d) for d in D_eff]
    off = np.concatenate([[0], np.cumsum([P * d for d in D_eff])]).astype(np.int64)

    nc = bacc.Bacc("TRN2", target_bir_lowering=False, debug=False,
                   num_devices=n_cores)
    kp = nc.declare_dram_parameter("kt", [128, S], BF16, isOutput=False)
    vp = nc.declare_dram_parameter("vt", [ROWS, S], BF16, isOutput=False)
    qp = nc.declare_dram_parameter("qt", [128, NG * P], BF16, isOutput=False)
    pcp = nc.declare_dram_parameter("pc", [128, NG * P // 16], F32, isOutput=False)
    cstp = nc.declare_dram_parameter("cst", [128, 352], BF16, isOutput=False)
    out = nc.declare_dram_parameter("out", [96, NG * P], F32, isOutput=True)

    mult = mybir.AluOpType.mult

    with tile.TileContext(nc) as tc:
        with tc.tile_pool(name="res", bufs=1) as res, \
             tc.tile_pool(name="kv", bufs=3) as kvp, \
             tc.tile_pool(name="work", bufs=2) as work, \
             tc.tile_pool(name="stg", bufs=2) as stg, \
             tc.psum_pool(name="pl", bufs=2) as plp, \
             tc.psum_pool(name="acc", bufs=2) as accp, \
             tc.psum_pool(name="rp", bufs=2) as rpp:
            qt_sb = res.tile([128, NG * P], BF16)
            nc.sync.dma_start(qt_sb[:], qp[:])
            cst_sb = res.tile([128, 352], BF16)
            nc.sync.dma_start(cst_sb[:], cstp[:])
            ones128 = cst_sb[:, 0:128]
            I128 = cst_sb[:, 128:256]
            rep8 = cst_sb[0:8, 256:352]

            out_sb = res.tile([ROWS, NG * P], F32)

            def emit_accums(rem, g):
                acc = accp.tile([128, P], F32, tag="acc")
                for i, (t, d) in enumerate(rem):
                    nc.tensor.matmul(
                        acc[0:ROWS, :], I128[0:ROWS, 0:ROWS],
                        t[0:ROWS, d * P:(d + 1) * P],
                        start=(i == 0), stop=(i == len(rem) - 1))
                return acc

            def emit_copy(acc, g):
                nc.scalar.copy(out_sb[:, g * P:(g + 1) * P], acc[0:ROWS, :])

            pending = None  # (rem, g) awaiting segment-sum matmuls
            pend_copy = None  # (acc, g) awaiting PSUM->SBUF copy

            def front(g):
                """DMA + kmul + m1 matmuls + exp for group g."""
                D = D_eff[g]
                C = D * P
                s0 = int(off[g])
                kt = kvp.tile([128, C], BF16, tag="kt")
                nc.sync.dma_start(kt[:], kp[:, s0:s0 + C])
                vt = kvp.tile([128, C], BF16, tag="vt")
                nc.sync.dma_start(vt[0:ROWS, :], vp[:, s0:s0 + C])

                w = work.tile([128, C], BF16, tag="w")
                w3 = w[:].rearrange("p (d f) -> p d f", d=D)
                kt3 = kt[:].rearrange("p (d f) -> p d f", d=D)
                qb = (qt_sb[:, g * P:(g + 1) * P]
                      .unsqueeze(1).broadcast_to([128, D, P]))
                nc.vector.tensor_tensor(out=w3[:], in0=kt3[:], in1=qb, op=mult)

                ew = work.tile([128, C], BF16, tag="ew")
                for c0 in range(0, C, EXP_CHUNK):
                    cw = min(EXP_CHUNK, C - c0)
                    pl = plp.tile([128, cw], F32, tag="pl")
                    for m0 in range(0, cw, 512):
                        mw = min(512, cw - m0)
                        nc.tensor.matmul(
                            pl[:, m0:m0 + mw], ones128,
                            w[:, c0 + m0:c0 + m0 + mw],
                            start=True, stop=True)
                    nc.scalar.activation(
                        out=ew[:, c0:c0 + cw], in_=pl[:],
                        func=mybir.ActivationFunctionType.Exp, scale=SCALE)
                return vt, ew

            def back(g, vt, ew):
                """vmul + d-halving tree for group g; flush older segment
                sums (PE matmuls lag so the in-order PE never stalls)."""
                nonlocal pending, pend_copy
                D = D_eff[g]
                C = D * P
                wv = work.tile([128, C], BF16, tag="wv")
                nc.vector.tensor_tensor(
                    out=wv[:], in0=vt[:, :C], in1=ew[:], op=mult)
                Dh = D // 2
                Ch = Dh * P
                wvh = work.tile([128, Ch], BF16, tag="wvh")
                nc.vector.tensor_tensor(
                    out=wvh[:, :Ch], in0=wv[:, :Ch], in1=wv[:, Ch:],
                    op=mybir.AluOpType.add)
                rem = [(wvh, d) for d in range(Dh)]
                if pending is not None:
                    pend_copy2 = (emit_accums(*pending), pending[1])
                    pending = None
                else:
                    pend_copy2 = None
                if pend_copy is not None:
                    emit_copy(*pend_copy)
                pend_copy = pend_copy2
                pending = (rem, g)

            prev = front(0)
            for g in range(NG):
                nxt = front(g + 1) if g + 1 < NG else None
                back(g, *prev)
                prev = nxt
            emit_copy(emit_accums(*pending), pending[1])
            if pend_copy is not None:
                emit_copy(*pend_copy)

            # ---- endgame: denominators -> reciprocal -> replicate -> scale
            # Reciprocal runs on the denominators packed onto all 128
            # partitions ([8, T] -> [128, T/16] via SBUF->SBUF DMA): the
            # iterative-divide DVE op is ~7 cyc/elem, 16x partition packing
            # makes it cheap.
            T = NG * P
            W = T // 16
            pc_sb = res.tile([128, W], F32)
            nc.sync.dma_start(pc_sb[:], pcp[:])
            dnp = res.tile([128, W], F32)
            for k in range(16):
                nc.sync.dma_start(dnp[k * 8:(k + 1) * 8, :],
                                  out_sb[96:104, k * W:(k + 1) * W])
            sbt = res.tile([128, W], F32)
            nc.vector.tensor_sub(out=sbt[:], in0=dnp[:], in1=pc_sb[:])
            rcp = res.tile([128, W], F32)
            nc.vector.reciprocal(out=rcp[:], in_=sbt[:])
            rcpb = res.tile([128, W], BF16)
            nc.vector.tensor_copy(rcpb[:], rcp[:])
            rcb = res.tile([8, T], BF16)
            for k in range(16):
                nc.sync.dma_start(rcb[:, k * W:(k + 1) * W],
                                  rcpb[k * 8:(k + 1) * 8, :])
            for c0 in range(0, T, 512):
                cw = min(512, T - c0)
                rp = rpp.tile([96, cw], F32, tag="rp")
                nc.tensor.matmul(rp[:], rep8, rcb[:, c0:c0 + cw],
                                 start=True, stop=True)
                st = stg.tile([96, cw], F32, tag="st")
                nc.vector.tensor_tensor(
                    out=st[:], in0=out_sb[0:96, c0:c0 + cw], in1=rp[:], op=mult)
                nc.sync.dma_start(out[:, c0:c0 + cw], st[:])

    nc.compile()
    return nc


# ---------------------------------------------------------------- entry point

LAST_RESULT = None


def kernel(value, key, query0, query1, edge_index):
    global LAST_RESULT
    import os
    in_maps, meta = prepare(value, key, query0, query1, edge_index)
    nc = build(meta["D_eff"], meta["S"], meta["NG"])
    res = run_bass_kernel_spmd(nc, in_maps, list(range(N_CORES)),
                               tmpdir=os.environ.get("BASS_SPMD_TMPDIR"))
    LAST_RESULT = res
    out_cores = [res.results[c]["out"] for c in range(N_CORES)]
    return unshard_output(out_cores, meta)


# revision 24
# speedup vs baseline: 1.0162x; 1.0162x over previous
"""Trainium2 Bass kernel for nn_AttentionSE3 (graph attention message passing).

v2 strategy (feature-on-partition transposed ELL layout, TensorE reductions):
- Host sorts nodes by in-degree into 128-node blocks, deals blocks round-robin
  to 8 cores, pads each block's edge lists to the block-group capacity D.
  Per group g the slots form a [D, 128] grid (d-major): col = d*128 + node_row.
- Device tiles are TRANSPOSED: kT [128 feats, D*128], vT [104, D*128] where
  rows 0..95 are value channels (h-major: c = h*12 + cx) and rows 96..103 are
  constant 1.0 (so the weighted-value product's rows 96..103 carry the raw
  exp-weights, giving the softmax denominator for free in the same matmul).
- Per group: DVE computes w = kT * q (q broadcast over d: stride-0 middle dim,
  contiguous 128-wide inner runs -> full 2x bf16 throughput).  TensorE reduces
  over the 16 k-features of each head AND replicates each head's logit to its
  13 output rows in one matmul with a fixed block-ones lhsT [128, 104].
  ScalarE applies exp (scale folded) PSUM->SBUF.  DVE multiplies by vT.
  TensorE then segment-sums over d via D accumulating identity matmuls into a
  [104, 128] PSUM tile (rows 0..95 weighted values, 96..103 denominators).
- Softmax max-subtraction is dropped (logits ~ N(0,1): exp never overflows);
  padded slots contribute exp(0)=1 to the denominator and are corrected by a
  host-computed pad count.  Normalization happens once at the end: denominator
  rows - pad counts, reciprocal, TensorE 8->96 replication, one multiply.
"""

import numpy as np

import concourse.bacc as bacc
import concourse.mybir as mybir
from concourse import tile
from concourse.bass_utils import run_bass_kernel_spmd

try:
    import ml_dtypes
    BF16_NP = np.dtype(ml_dtypes.bfloat16)
except ImportError:  # pragma: no cover
    BF16_NP = None

N_NODES = 50000
H = 8
P = 128  # nodes per block
N_CORES = 8
ROWS = 104  # 96 value channels + 8 ones-rows (denominator trick)
SCALE = float(1.0 / np.sqrt(128.0))
F32 = mybir.dt.float32
BF16 = mybir.dt.bfloat16

# Fraction of the two big elementwise multiplies routed to GPSIMD.
GP_FRAC_K = 0.0
GP_FRAC_V = 0.0
GP_FRAC_T = 0.0  # share of the d-halving tree pass on GPSIMD
# ScalarE exp chunk width (PSUM tile cols; matmuls within are <=512)
EXP_CHUNK = 1024


# ---------------------------------------------------------------- host prep

def prepare(value, key, query0, query1, edge_index, n_nodes=N_NODES, n_cores=N_CORES):
    value = np.asarray(value, dtype=np.float32)
    key = np.asarray(key, dtype=np.float32)
    query0 = np.asarray(query0, dtype=np.float32)
    query1 = np.asarray(query1, dtype=np.float32)
    n_edges = key.shape[0]

    dst = np.asarray(edge_index[1], dtype=np.int64)
    deg = np.bincount(dst, minlength=n_nodes).astype(np.int64)
    n_pad = -(-n_nodes // (P * n_cores)) * (P * n_cores)
    deg_pad = np.concatenate([deg, np.zeros(n_pad - n_nodes, dtype=np.int64)])
    nb = n_pad // P
    ng = nb // n_cores

    order = np.argsort(deg_pad, kind="stable")  # node ids, degree-ascending
    degs_o = deg_pad[order]

    blk_max = degs_o.reshape(nb, P).max(axis=1)
    D_eff = np.maximum(blk_max.reshape(ng, n_cores).max(axis=1), 1).astype(np.int64)
    D_eff = (D_eff + 1) // 2 * 2  # even, for the d-halving tree pass
    off = np.concatenate([[0], np.cumsum(P * D_eff)]).astype(np.int64)
    S = int(off[-1])  # cols per core

    pos = np.arange(n_pad)
    block = pos // P
    g_of = block // n_cores
    core_of = block % n_cores
    row = pos % P

    edge_order = np.argsort(dst, kind="stable")
    starts = np.concatenate([[0], np.cumsum(deg)])

    pp = np.repeat(pos, degs_o)           # padded-node position per real edge
    cum0 = np.concatenate([[0], np.cumsum(degs_o)])[:-1]
    d_idx = np.arange(n_edges) - np.repeat(cum0, degs_o)
    node_of_pp = order[pp]
    edge_ids = edge_order[starts[node_of_pp] + d_idx]
    # d-major slot layout: col = off[g] + d*128 + row
    col_global = core_of[pp] * S + off[g_of[pp]] + d_idx * P + row[pp]

    dt = BF16_NP
    kp_flat = np.zeros((n_cores * S, 128), dtype=dt)
    kp_flat[col_global] = key[edge_ids]
    vp_flat = np.zeros((n_cores * S, 96), dtype=dt)
    vp_flat[col_global] = value.reshape(n_edges, 96)[edge_ids]

    qfull = np.concatenate([query0, query1], axis=-1).reshape(n_nodes, 128)
    q_pad = np.zeros((n_pad, 128), dtype=np.float32)
    q_pad[:n_nodes] = qfull

    pc = (D_eff[g_of] - degs_o[pos]).astype(np.float32)  # pad count per padded node
    zero_deg = degs_o[pos] == 0
    pc[zero_deg] = (D_eff[g_of[zero_deg]] - 1).astype(np.float32)

    ids_blocks = order.reshape(nb, P)

    in_maps = []
    for c in range(n_cores):
        kT = np.ascontiguousarray(kp_flat[c * S:(c + 1) * S].T)  # [128, S]
        vT = np.empty((ROWS, S), dtype=dt)
        vT[:96] = vp_flat[c * S:(c + 1) * S].T
        vT[96:] = np.ones((8, S), dtype=dt)
        ids_c = ids_blocks[c::n_cores]                           # [ng, 128]
        qT = np.ascontiguousarray(
            q_pad[ids_c].transpose(2, 0, 1).reshape(128, ng * P)).astype(dt)
        # pad counts packed [8, ng*P] -> [128, ng*P//16] (partition kp = k*8+p
        # holds wide[p, k*W + j]); rows p identical so only k*W+j matters.
        pc_row = pc.reshape(nb, P)[c::n_cores].reshape(ng * P)
        W = ng * P // 16
        pc_c = np.ascontiguousarray(
            np.broadcast_to(pc_row.reshape(16, 1, W), (16, 8, W))
            .reshape(128, W)).astype(np.float32)
        in_maps.append({"kt": kT, "vt": vT, "qt": qT, "pc": pc_c,
                        "cst": _make_consts()})

    meta = dict(D_eff=D_eff, off=off, S=S, NG=ng, NB=nb, order=order,
                n_nodes=n_nodes, n_pad=n_pad)
    return in_maps, meta


def _make_consts():
    """lhsT constants [128, 352] bf16: block-ones [128,128] | I128 | rep8->96.
    ones128 cols 104..127 are zero so the padded output rows are exact 0."""
    cst = np.zeros((128, 352), dtype=BF16_NP)
    pidx = np.arange(128)
    hp = pidx // 16
    for c in range(104):
        hc = c // 12 if c < 96 else c - 96
        cst[hp == hc, c] = 1.0
    cst[:, 128:256] = np.eye(128, dtype=np.float32)
    for c in range(96):
        cst[c // 12, 256 + c] = 1.0
    return cst


def unshard_output(out_cores, meta):
    """out_cores: list of [96, NG*128] f32 -> [n_nodes, 32, 3]."""
    ng, nb = meta["NG"], meta["NB"]
    n_cores = len(out_cores)
    order, n_nodes, n_pad = meta["order"], meta["n_nodes"], meta["n_pad"]
    out_sorted = np.zeros((nb, P, 96), dtype=np.float32)
    for c in range(n_cores):
        out_sorted[c::n_cores] = (
            out_cores[c].reshape(96, ng, P).transpose(1, 2, 0))
    out_sorted = out_sorted.reshape(n_pad, 96)
    out_full = np.zeros((n_nodes, 96), dtype=np.float32)
    mask = order < n_nodes
    out_full[order[mask]] = out_sorted[mask]
    return out_full.reshape(n_nodes, 32, 3)


# ---------------------------------------------------------------- bass kernel

def build(D_eff, S, NG, n_cores=N_CORES):
    D_eff = [int(d) for d in D_eff]
    off = np.concatenate([[0], np.cumsum([P * d for d in D_eff])]).astype(np.int64)

    nc = bacc.Bacc("TRN2", target_bir_lowering=False, debug=False,
                   num_devices=n_cores)
    kp = nc.declare_dram_parameter("kt", [128, S], BF16, isOutput=False)
    vp = nc.declare_dram_parameter("vt", [ROWS, S], BF16, isOutput=False)
    qp = nc.declare_dram_parameter("qt", [128, NG * P], BF16, isOutput=False)
    pcp = nc.declare_dram_parameter("pc", [128, NG * P // 16], F32, isOutput=False)
    cstp = nc.declare_dram_parameter("cst", [128, 352], BF16, isOutput=False)
    out = nc.declare_dram_parameter("out", [96, NG * P], F32, isOutput=True)

    mult = mybir.AluOpType.mult

    with tile.TileContext(nc) as tc:
        with tc.tile_pool(name="res", bufs=1) as res, \
             tc.tile_pool(name="kv", bufs=3) as kvp, \
             tc.tile_pool(name="work", bufs=2) as work, \
             tc.tile_pool(name="stg", bufs=2) as stg, \
             tc.psum_pool(name="pl", bufs=2) as plp, \
             tc.psum_pool(name="acc", bufs=2) as accp, \
             tc.psum_pool(name="rp", bufs=2) as rpp:
            qt_sb = res.tile([128, NG * P], BF16)
            nc.sync.dma_start(qt_sb[:], qp[:])
            cst_sb = res.tile([128, 352], BF16)
            nc.sync.dma_start(cst_sb[:], cstp[:])
            ones128 = cst_sb[:, 0:128]
            I128 = cst_sb[:, 128:256]
            rep8 = cst_sb[0:8, 256:352]

            out_sb = res.tile([ROWS, NG * P], F32)

            def emit_accums(rem, g):
                acc = accp.tile([128, P], F32, tag="acc")
                for i, (t, d) in enumerate(rem):
                    nc.tensor.matmul(
                        acc[0:ROWS, :], I128[0:ROWS, 0:ROWS],
                        t[0:ROWS, d * P:(d + 1) * P],
                        start=(i == 0), stop=(i == len(rem) - 1))
                return acc

            def emit_copy(acc, g):
                nc.scalar.copy(out_sb[:, g * P:(g + 1) * P], acc[0:ROWS, :])

            pending = None  # (rem, g) awaiting segment-sum matmuls
            pend_copy = None  # (acc, g) awaiting PSUM->SBUF copy

            def front(g):
                """DMA + kmul + m1 matmuls + exp for group g."""
                D = D_eff[g]
                C = D * P
                s0 = int(off[g])
                kt = kvp.tile([128, C], BF16, tag="kt")
                nc.sync.dma_start(kt[:], kp[:, s0:s0 + C])
                vt = kvp.tile([128, C], BF16, tag="vt")
                nc.sync.dma_start(vt[0:ROWS, :], vp[:, s0:s0 + C])

                w = work.tile([128, C], BF16, tag="w")
                w3 = w[:].rearrange("p (d f) -> p d f", d=D)
                kt3 = kt[:].rearrange("p (d f) -> p d f", d=D)
                qb = (qt_sb[:, g * P:(g + 1) * P]
                      .unsqueeze(1).broadcast_to([128, D, P]))
                nc.vector.tensor_tensor(out=w3[:], in0=kt3[:], in1=qb, op=mult)

                ew = work.tile([128, C], BF16, tag="ew")
                for c0 in range(0, C, EXP_CHUNK):
                    cw = min(EXP_CHUNK, C - c0)
                    pl = plp.tile([128, cw], F32, tag="pl")
                    for m0 in range(0, cw, 512):
                        mw = min(512, cw - m0)
                        nc.tensor.matmul(
                            pl[:, m0:m0 + mw], ones128,
                            w[:, c0 + m0:c0 + m0 + mw],
                            start=True, stop=True)
                    nc.scalar.activation(
                        out=ew[:, c0:c0 + cw], in_=pl[:],
                        func=mybir.ActivationFunctionType.Exp, scale=SCALE)
                return vt, ew

            def back(g, vt, ew):
                """vmul + d-halving tree for group g; flush older segment
                sums (PE matmuls lag so the in-order PE never stalls)."""
                nonlocal pending, pend_copy
                D = D_eff[g]
                C = D * P
                wv = work.tile([128, C], BF16, tag="wv")
                nc.vector.tensor_tensor(
                    out=wv[:], in0=vt[:, :C], in1=ew[:], op=mult)
                Dh = D // 2
                Ch = Dh * P
                wvh = work.tile([128, Ch], BF16, tag="wvh")
                nc.vector.tensor_tensor(
                    out=wvh[:, :Ch], in0=wv[:, :Ch], in1=wv[:, Ch:],
                    op=mybir.AluOpType.add)
                Dq = Dh // 2
                if Dq > 0:
                    Cq = Dq * P
                    wvq = work.tile([128, Cq], BF16, tag="wvq")
                    nc.vector.tensor_tensor(
                        out=wvq[:, :Cq], in0=wvh[:, :Cq],
                        in1=wvh[:, Cq:2 * Cq], op=mybir.AluOpType.add)
                    rem = [(wvq, d) for d in range(Dq)]
                    if Dh % 2 == 1:
                        rem.append((wvh, Dh - 1))
                else:
                    rem = [(wvh, 0)]
                if pending is not None:
                    pend_copy2 = (emit_accums(*pending), pending[1])
                    pending = None
                else:
                    pend_copy2 = None
                if pend_copy is not None:
                    emit_copy(*pend_copy)
                pend_copy = pend_copy2
                pending = (rem, g)

            prev = front(0)
            for g in range(NG):
                nxt = front(g + 1) if g + 1 < NG else None
                back(g, *prev)
                prev = nxt
            emit_copy(emit_accums(*pending), pending[1])
            if pend_copy is not None:
                emit_copy(*pend_copy)

            # ---- endgame: denominators -> reciprocal -> replicate -> scale
            # Reciprocal runs on the denominators packed onto all 128
            # partitions ([8, T] -> [128, T/16] via SBUF->SBUF DMA): the
            # iterative-divide DVE op is ~7 cyc/elem, 16x partition packing
            # makes it cheap.
            T = NG * P
            W = T // 16
            pc_sb = res.tile([128, W], F32)
            nc.sync.dma_start(pc_sb[:], pcp[:])
            dnp = res.tile([128, W], F32)
            for k in range(16):
                nc.sync.dma_start(dnp[k * 8:(k + 1) * 8, :],
                                  out_sb[96:104, k * W:(k + 1) * W])
            sbt = res.tile([128, W], F32)
            nc.vector.tensor_sub(out=sbt[:], in0=dnp[:], in1=pc_sb[:])
            rcp = res.tile([128, W], F32)
            nc.vector.reciprocal(out=rcp[:], in_=sbt[:])
            rcpb = res.tile([128, W], BF16)
            nc.vector.tensor_copy(rcpb[:], rcp[:])
            rcb = res.tile([8, T], BF16)
            for k in range(16):
                nc.sync.dma_start(rcb[:, k * W:(k + 1) * W],
                                  rcpb[k * 8:(k + 1) * 8, :])
            for c0 in range(0, T, 512):
                cw = min(512, T - c0)
                rp = rpp.tile([96, cw], F32, tag="rp")
                nc.tensor.matmul(rp[:], rep8, rcb[:, c0:c0 + cw],
                                 start=True, stop=True)
                st = stg.tile([96, cw], F32, tag="st")
                nc.vector.tensor_tensor(
                    out=st[:], in0=out_sb[0:96, c0:c0 + cw], in1=rp[:], op=mult)
                nc.sync.dma_start(out[:, c0:c0 + cw], st[:])

    nc.compile()
    return nc


# ---------------------------------------------------------------- entry point

LAST_RESULT = None


def kernel(value, key, query0, query1, edge_index):
    global LAST_RESULT
    import os
    in_maps, meta = prepare(value, key, query0, query1, edge_index)
    nc = build(meta["D_eff"], meta["S"], meta["NG"])
    res = run_bass_kernel_spmd(nc, in_maps, list(range(N_CORES)),
                               tmpdir=os.environ.get("BASS_SPMD_TMPDIR"))
    LAST_RESULT = res
    out_cores = [res.results[c]["out"] for c in range(N_CORES)]
    return unshard_output(out_cores, meta)
